# revision 21
# baseline (speedup 1.0000x reference)
import sys

sys.path.insert(0, "/opt/trn_rl_repo")

import numpy as np

import concourse.bass as bass
import concourse.mybir as mybir
import concourse.tile as tile
from concourse import bacc
from concourse.bass_utils import run_bass_kernel_spmd

B, L, H, T = 512, 30, 1024, 30
NC = 8
BS = B // NC          # 64 batch rows per core
TB = T * BS           # 1920 (t-major columns: col = t*BS + b)
BL = BS * L           # 1920 (b-major columns: col = b*L + l)
H3 = 3 * H            # 3072
KC = H // 128         # 8 contraction chunks
MT = H3 // 128        # 24 output row-tiles of gi/gh

F32 = mybir.dt.float32
AF = mybir.ActivationFunctionType
ALU = mybir.AluOpType
AX = mybir.AxisListType
NEG = -1.0e30

LAST_RESULT = None


def build_core(trace=False):
    nc = bacc.Bacc(None, target_bir_lowering=False)

    xst = nc.dram_tensor("xst", [H, TB], F32, kind="ExternalInput")
    iht = nc.dram_tensor("iht", [H, BL], F32, kind="ExternalInput")
    h0p = nc.dram_tensor("h0p", [128, KC * BS], F32, kind="ExternalInput")
    wiht = nc.dram_tensor("wiht", [H, H3], F32, kind="ExternalInput")
    whht = nc.dram_tensor("whht", [H, H3], F32, kind="ExternalInput")
    wlt = nc.dram_tensor("wlt", [H, H], F32, kind="ExternalInput")
    bsum = nc.dram_tensor("bsum", [H3, 1], F32, kind="ExternalInput")
    bhn = nc.dram_tensor("bhn", [128, 512], F32, kind="ExternalInput")
    wlb = nc.dram_tensor("wlb", [H, 1], F32, kind="ExternalInput")
    mask0 = nc.dram_tensor("mask0", [BS, L], F32, kind="ExternalInput")

    giT = nc.dram_tensor("giT", [H3, TB], F32, kind="Internal")

    preds = nc.dram_tensor("preds", [BS, T, L], F32, kind="ExternalOutput")
    inf = nc.dram_tensor("inf", [BS, T, L], F32, kind="ExternalOutput")

    with tile.TileContext(nc) as tc:
        # ---- Phase A: giT = W_ih @ XS.T + bsum  (transposed, all steps) ----
        # giT[mi*128+p, t*64+b] = gi[(t,b), mi*128+p] + (b_ih+b_hh)[mi*128+p]
        with tc.tile_pool(name="a_sb", bufs=1) as pa, \
             tc.tile_pool(name="a_st", bufs=3) as pst, \
             tc.tile_pool(name="a_ps", bufs=2, space="PSUM") as pps:
            xst_sb = [pa.tile([128, TB], F32, name=f"xst{k}") for k in range(KC)]
            wiht_sb = [pa.tile([128, H3], F32, name=f"wih{k}") for k in range(KC)]
            bsum_sb = pa.tile([128, MT], F32, name="bsum_sb")
            for k in range(KC):
                nc.sync.dma_start(xst_sb[k][:, :], xst[k * 128:(k + 1) * 128, :])
                nc.sync.dma_start(wiht_sb[k][:, :], wiht[k * 128:(k + 1) * 128, :])
            for mi in range(MT):
                nc.sync.dma_start(bsum_sb[:, mi:mi + 1],
                                  bsum[mi * 128:(mi + 1) * 128, :])
            for mi in range(MT):                # 24 row tiles of 128
                for n in range(4):              # 4 col chunks of 480
                    ps = pps.tile([128, 480], F32, name="gi_acc")
                    for k in range(KC):
                        nc.tensor.matmul(
                            ps[:, :],
                            wiht_sb[k][:, mi * 128:(mi + 1) * 128],
                            xst_sb[k][:, n * 480:(n + 1) * 480],
                            start=(k == 0), stop=(k == KC - 1))
                    st = pst.tile([128, 480], F32, name="gi_st")
                    nc.scalar.activation(st[:, :], ps[:, :], AF.Identity,
                                         bias=bsum_sb[:, mi:mi + 1])
                    nc.sync.dma_start(
                        giT[mi * 128:(mi + 1) * 128, n * 480:(n + 1) * 480],
                        st[:, :])

        # ---- Phases B-D under shared pools --------------------------------
        with tc.tile_pool(name="hall_p", bufs=1) as phall:
            # hall[p, t*512 + k*64 + b] = h_t[b, k*128+p]
            hall = phall.tile([128, T * 512], F32, name="hall")

            # ---- Phase B: 30-step GRU, transposed/packed layout -----------
            with tc.tile_pool(name="b_sb", bufs=1) as pb, \
                 tc.tile_pool(name="b_gi", bufs=2) as pgi, \
                 tc.tile_pool(name="b_gt", bufs=1) as pgt, \
                 tc.tile_pool(name="b_ps", bufs=2, space="PSUM") as pbps:
                whht_sb = [pb.tile([128, H3], F32, name=f"whh{k}")
                           for k in range(KC)]
                h0p_sb = pb.tile([128, KC * BS], F32, name="h0p_sb")
                bhn_sb = pb.tile([128, 512], F32, name="bhn_sb")
                nc.sync.dma_start(bhn_sb[:, :], bhn[:, :])
                r_t = pgt.tile([128, 512], F32, name="r_t")
                z_t = pgt.tile([128, 512], F32, name="z_t")
                n_t = pgt.tile([128, 512], F32, name="n_t")
                t1 = pgt.tile([128, 512], F32, name="t1")
                for k in range(KC):
                    nc.sync.dma_start(whht_sb[k][:, :],
                                      whht[k * 128:(k + 1) * 128, :])
                nc.sync.dma_start(h0p_sb[:, :], h0p[:, :])

                for t in range(T):
                    git = pgi.tile([128, 3 * 512], F32, name="git")
                    git3 = git[:, :].rearrange("p (g m b) -> p g m b",
                                               g=3, m=KC)
                    for g in range(3):
                        nc.sync.dma_start(
                            git3[:, g, :, :],
                            giT[g * H:(g + 1) * H, t * BS:(t + 1) * BS]
                            .rearrange("(m p) b -> p m b", m=KC))
                    hprev = (h0p_sb[:, :] if t == 0
                             else hall[:, (t - 1) * 512:t * 512])
                    gps = []
                    for g in (0, 2, 1):         # r, n, z tile order
                        ps = pbps.tile([128, 512], F32, name=f"gh{g}")
                        gps.append((g, ps))
                        for ml in range(8):
                            mi = g * 8 + ml
                            sl = slice(ml * BS, (ml + 1) * BS)
                            for k in range(KC):
                                nc.tensor.matmul(
                                    ps[:, sl],
                                    whht_sb[k][:, mi * 128:(mi + 1) * 128],
                                    hprev[:, k * BS:(k + 1) * BS],
                                    start=(k == 0), stop=(k == KC - 1))
                    ghr = gps[0][1]
                    ghn = gps[1][1]
                    ghz = gps[2][1]
                    # r = sigmoid(gi_r + gh_r)
                    nc.vector.tensor_add(r_t[:, :], git[:, 0:512], ghr[:, :])
                    nc.scalar.activation(r_t[:, :], r_t[:, :], AF.Sigmoid)
                    # n = tanh(gi_n + r * (gh_n + b_hh_n))
                    nc.vector.tensor_add(n_t[:, :], bhn_sb[:, :], ghn[:, :])
                    nc.vector.tensor_mul(n_t[:, :], r_t[:, :], n_t[:, :])
                    nc.vector.tensor_add(n_t[:, :], n_t[:, :],
                                         git[:, 1024:1536])
                    nc.scalar.activation(n_t[:, :], n_t[:, :], AF.Tanh)
                    # z = sigmoid(gi_z + gh_z)
                    nc.vector.tensor_add(z_t[:, :], git[:, 512:1024],
                                         ghz[:, :])
                    nc.scalar.activation(z_t[:, :], z_t[:, :], AF.Sigmoid)
                    # h = n + z * (hprev - n)
                    hcur = hall[:, t * 512:(t + 1) * 512]
                    nc.vector.tensor_sub(t1[:, :], hprev, n_t[:, :])
                    nc.vector.tensor_mul(t1[:, :], z_t[:, :], t1[:, :])
                    nc.vector.tensor_add(hcur, n_t[:, :], t1[:, :])

            # ---- Phase C: qT = Wl_w @ hall + Wl_b -------------------------
            with tc.tile_pool(name="q_sb", bufs=1) as pq:
                qt_sb = [pq.tile([128, TB], F32, name=f"qt{m}")
                         for m in range(KC)]
                with tc.tile_pool(name="c_sb", bufs=1) as pc, \
                     tc.tile_pool(name="c_ps", bufs=2, space="PSUM") as pcps:
                    wlt_sb = [pc.tile([128, H], F32, name=f"wl{k}")
                              for k in range(KC)]
                    wlb_sb = pc.tile([128, KC], F32, name="wlb_sb")
                    for k in range(KC):
                        nc.sync.dma_start(wlt_sb[k][:, :],
                                          wlt[k * 128:(k + 1) * 128, :])
                        nc.sync.dma_start(wlb_sb[:, k:k + 1],
                                          wlb[k * 128:(k + 1) * 128, :])
                    hall4 = hall[:, :].rearrange("p (t k b) -> p t k b",
                                                 t=T, k=KC)
                    for m in range(KC):         # 8 output-dim tiles of 128
                        for n in range(4):      # t-block chunks of 8,8,8,6
                            nt = 8 if n < 3 else 6
                            ps = pcps.tile([128, 512], F32, name="q_acc")
                            for k in range(KC):
                                nc.tensor.matmul(
                                    ps[:, 0:nt * BS],
                                    wlt_sb[k][:, m * 128:(m + 1) * 128],
                                    hall4[:, n * 8:n * 8 + nt, k, :],
                                    start=(k == 0), stop=(k == KC - 1))
                            nc.scalar.activation(
                                qt_sb[m][:, n * 512:n * 512 + nt * BS],
                                ps[:, 0:nt * BS], AF.Identity,
                                bias=wlb_sb[:, m:m + 1])

                # ---- Phase D: pre[b] = Q_b @ IH_b.T  (per batch row) ------
                with tc.tile_pool(name="d_sb", bufs=1) as pd, \
                     tc.tile_pool(name="d_st", bufs=4) as pdst, \
                     tc.tile_pool(name="d_ps", bufs=4, space="PSUM") as pdps:
                    iht_sb = [pd.tile([128, BL], F32, name=f"ih{k}")
                              for k in range(KC)]
                    for k in range(KC):
                        nc.sync.dma_start(iht_sb[k][:, :],
                                          iht[k * 128:(k + 1) * 128, :])
                    for b in range(BS):
                        ps = pdps.tile([T, L], F32, name="pre_acc")
                        for k in range(KC):
                            nc.tensor.matmul(
                                ps[:, :],
                                qt_sb[k][:, b:b + (T - 1) * BS + 1:BS],
                                iht_sb[k][:, b * L:(b + 1) * L],
                                start=(k == 0), stop=(k == KC - 1))
                        pst_b = pdst.tile([T, L], F32, name="pre_st")
                        nc.vector.tensor_copy(pst_b[:, :], ps[:, :])
                        nc.sync.dma_start(preds[b, :, :], pst_b[:, :])

        # ---- Phase E: masked softmax + greedy flag chain ------------------
        with tc.tile_pool(name="e_sb", bufs=1) as pe:
            pre_sb = pe.tile([BS, T * L], F32, name="pre_sb")
            inf_sb = pe.tile([BS, T * L], F32, name="inf_sb")
            madd = pe.tile([BS, L], F32, name="madd")
            msk = pe.tile([BS, L], F32, name="msk")
            e_t = pe.tile([BS, L], F32, name="e_t")
            oh = pe.tile([BS, L], F32, name="oh")
            mx = pe.tile([BS, 1], F32, name="mx")
            nmx = pe.tile([BS, 1], F32, name="nmx")
            ssum = pe.tile([BS, 1], F32, name="ssum")
            rec = pe.tile([BS, 1], F32, name="rec")
            nc.sync.dma_start(pre_sb[:, :],
                              preds[:, :, :].rearrange("b t l -> b (t l)"))
            nc.sync.dma_start(madd[:, :], mask0[:, :])
            for t in range(T):
                sl = slice(t * L, (t + 1) * L)
                nc.vector.tensor_add(msk[:, :], pre_sb[:, sl], madd[:, :])
                nc.vector.tensor_reduce(mx[:, :], msk[:, :], axis=AX.X, op=ALU.max)
                nc.vector.tensor_scalar_mul(nmx[:, :], mx[:, :], -1.0)
                nc.scalar.activation(e_t[:, :], msk[:, :], AF.Exp,
                                     bias=nmx[:, 0:1], accum_out=ssum[:, :])
                nc.vector.reciprocal(rec[:, :], ssum[:, :])
                nc.vector.tensor_scalar_mul(inf_sb[:, sl], e_t[:, :], rec[:, 0:1])
                nc.vector.tensor_scalar(oh[:, :], msk[:, :], mx[:, 0:1], None,
                                        op0=ALU.is_equal)
                nc.vector.tensor_scalar(oh[:, :], oh[:, :], NEG, None, op0=ALU.mult)
                nc.vector.tensor_add(madd[:, :], madd[:, :], oh[:, :])
            nc.sync.dma_start(inf[:, :, :].rearrange("b t l -> b (t l)"),
                              inf_sb[:, :])

    nc.compile()
    return nc


def make_in_maps(input_hidden, tgt, pos_label, S, W_ih, W_hh, b_ih, b_hh, Wl_w, Wl_b):
    f = np.float32
    input_hidden = np.asarray(input_hidden, f)
    tgt = np.asarray(tgt, f)
    pos_label = np.asarray(pos_label)
    S = np.asarray(S, f)
    WIHT = np.ascontiguousarray(np.asarray(W_ih, f).T)
    WHHT = np.ascontiguousarray(np.asarray(W_hh, f).T)
    WLT = np.ascontiguousarray(np.asarray(Wl_w, f).T)
    b_ih = np.asarray(b_ih, f)
    b_hh = np.asarray(b_hh, f)
    # r,z thirds get b_ih+b_hh folded into gi; n third gets b_ih only
    # (b_hh_n is multiplied by r inside the cell, added separately via bhn)
    bsum = np.ascontiguousarray(
        np.concatenate([b_ih[:2 * H] + b_hh[:2 * H], b_ih[2 * H:]])
        .reshape(H3, 1))
    # bhn[p, ml*64+b] = b_hh[2H + ml*128 + p]
    bhn = np.ascontiguousarray(np.broadcast_to(
        b_hh[2 * H:].reshape(KC, 128).T[:, :, None],
        (128, KC, BS)).reshape(128, KC * BS))
    wlb = np.ascontiguousarray(np.asarray(Wl_b, f).reshape(H, 1))
    in_maps = []
    for c in range(NC):
        s = slice(c * BS, (c + 1) * BS)
        ih = input_hidden[s]
        xs = np.concatenate(
            [np.broadcast_to(S, (BS, 1, H)), tgt[s][:, :T - 1]], axis=1)
        xst = np.ascontiguousarray(xs.transpose(2, 1, 0).reshape(H, TB))
        ihtm = np.ascontiguousarray(ih.transpose(2, 0, 1).reshape(H, BL))
        h0 = ih.mean(axis=1, dtype=np.float32).astype(f)
        # h0p[p, k*64+b] = h0[b, k*128+p]
        h0p = np.ascontiguousarray(
            h0.T.reshape(KC, 128, BS).transpose(1, 0, 2).reshape(128, KC * BS))
        ss = (pos_label[s] != -1).sum(axis=1)
        validm = np.arange(L)[None, :] < ss[:, None]
        m0 = np.where(validm, np.float32(0.0), np.float32(NEG)).astype(f)
        in_maps.append(dict(
            xst=xst, iht=ihtm, h0p=h0p,
            wiht=WIHT, whht=WHHT, wlt=WLT, bsum=bsum, bhn=bhn, wlb=wlb,
            mask0=np.ascontiguousarray(m0)))
    return in_maps


_NC_CACHE = {}


def kernel(input_hidden, tgt, pos_label, max_length, S, W_ih, W_hh, b_ih, b_hh,
           Wl_w, Wl_b, trace=False):
    global LAST_RESULT
    assert int(max_length) == T
    if "nc" not in _NC_CACHE:
        _NC_CACHE["nc"] = build_core()
    nc = _NC_CACHE["nc"]
    in_maps = make_in_maps(input_hidden, tgt, pos_label, S, W_ih, W_hh,
                           b_ih, b_hh, Wl_w, Wl_b)
    res = run_bass_kernel_spmd(nc, in_maps, core_ids=list(range(NC)), trace=trace)
    LAST_RESULT = res
    outs = np.concatenate([r["preds"] for r in res.results], axis=0)
    infs = np.concatenate([r["inf"] for r in res.results], axis=0)
    return outs, infs
